# revision 1
# baseline (speedup 1.0000x reference)
"""DualMultiCopyGenerator - Trainium2 Bass kernel, 8 NeuronCores (SPMD).

Sharding: the extended vocab axis (VEXT = V + S1 + S2 = 32512) is split 8 ways
(4064 columns per core), so the big Wfc weight is read once across the chip and
each core produces its [1024, 4064] slice of the final blended output; the host
concatenates slices. Attention is sharded one (batch, source) pair per core,
with two tiny AllGathers (p-logit partials, scaled/transposed copy-attention
rows) and two tiny AllReduces (fc row sum-of-squares for the layer norm, one
per row half so the reduce overlaps compute).

Key transformations (exact up to bf16 rounding):
  - Wfc columns are mean-centered on host; layer_norm is shift-invariant per
    row, so fc row means become exactly 0 and only sumsq needs the reduce.
  - The copy scatter is a one-hot matmul: host compacts map indices per
    (core, batch) into <= K_pad slots; G[slot, col] is generated on device via
    iota + is_equal; duplicate indices accumulate exactly via G row collisions.
  - softmax(att) @ v @ Wo^T @ Wp_c^T collapses per head to
    (sum_s exp * (x_src @ A_h)) / (sum_s exp), with A_h = Wv_h^T Wo^T Wp_c^T
    fused on host; q/k are produced in transposed layout from host-transposed
    weights so no transposes sit on the scores path.
  - layer_norm of the copy-attention rows is scale-invariant, so the
    1/sqrt(dh) and 1/H factors drop; masked rows of q/k are exactly zero by
    construction so qmask/kmask only need the softmax-denominator correction
    (folded into the per-head ones column of A).
  - Final blend per chunk: PSUM accumulates the scatter matmul plus
    diag(p0/sigma) @ fc, and one ScalarE copy drains PSUM->SBUF for the
    output DMA.
"""
import sys
sys.path.insert(0, '/opt/trn_rl_repo')
import numpy as np
import ml_dtypes
import jax
import jax.numpy as jnp
from jax.sharding import Mesh, NamedSharding, PartitionSpec
from jax.experimental.shard_map import shard_map
import concourse.bacc as bacc
import concourse.mybir as mybir
from concourse import tile
from concourse import bass2jax
from contextlib import ExitStack

N_CORES = 8
B, T = 4, 256
D = 512
V = 32000
SB = 256                       # S1 == S2
VEXT = V + 2 * SB              # 32512
VSH = VEXT // N_CORES          # 4064
NROW = B * T                   # 1024
RT = NROW // 128               # 8 row tiles
CH = 8                         # vocab chunks per core
CW = VSH // CH                 # 508
KT = D // 128                  # 4
H, DH = 8, 64

F32 = mybir.dt.float32
BF16 = mybir.dt.bfloat16
AF = mybir.ActivationFunctionType
ALU = mybir.AluOpType
BF = ml_dtypes.bfloat16

_CACHE = {}


def _rsqrt_cols(nc, small, t_ap, inv_n, eps, tag):
    """r = 1/sqrt(t_ap*inv_n + eps), one Newton step (ACT Sqrt is low-precision)."""
    n = t_ap.shape[-1]
    tv = small.tile([128, n], F32, tag=tag + "tv")
    nc.vector.tensor_scalar(out=tv[:], in0=t_ap, scalar1=float(inv_n),
                            scalar2=float(eps), op0=ALU.mult, op1=ALU.add)
    sq = small.tile([128, n], F32, tag=tag + "sq")
    nc.scalar.activation(sq[:], tv[:], AF.Sqrt)
    r = small.tile([128, n], F32, tag=tag + "r")
    nc.vector.reciprocal(r[:], sq[:])
    e = small.tile([128, n], F32, tag=tag + "e")
    nc.vector.tensor_tensor(out=e[:], in0=r[:], in1=r[:], op=ALU.mult)
    nc.vector.tensor_tensor(out=e[:], in0=e[:], in1=tv[:], op=ALU.mult)
    nc.vector.tensor_scalar(out=e[:], in0=e[:], scalar1=-0.5, scalar2=1.5,
                            op0=ALU.mult, op1=ALU.add)
    nc.vector.tensor_tensor(out=r[:], in0=r[:], in1=e[:], op=ALU.mult)
    return r


def build_program(kp_t, stage=5, reps=1, no_coll=False):
    nc = bacc.Bacc("TRN2", target_bir_lowering=False, debug=False,
                   num_devices=N_CORES)

    def din(name, shape, dt=BF16):
        return nc.dram_tensor(name, shape, dt, kind="ExternalInput").ap()

    xT = din("xT", [128, KT * NROW])
    Wsw = din("Wsw", [CH, 128, KT * CW])
    xqT = din("xqT", [128, KT * T])
    srcT = din("srcT", [128, KT * SB])
    WqT = din("WqT", [128, KT * D])
    WkT = din("WkT", [128, KT * D])
    Amat = din("Amat", [128, KT * 32])
    WpxT = din("WpxT", [128, KT * 3])
    kmask = din("kmask", [128, 2], F32)
    bsel = din("bsel", [128, B * 2], F32)
    EInv = din("EInv", [128, B * 4 * kp_t], F32)
    Gcol = din("Gcol", [128, B * kp_t], F32)
    out = nc.dram_tensor("out", [NROW, VSH], F32, kind="ExternalOutput").ap()

    GROUPS = [(0, 6), (6, 2)]  # (first rowtile, count): async stats per group
    st_in = [nc.dram_tensor(f"st_in{g}", [128, n], F32)
             for g, (_, n) in enumerate(GROUPS)]
    st_out = [nc.dram_tensor(f"st_out{g}", [128, n], F32, addr_space="Shared")
              for g, (_, n) in enumerate(GROUPS)]
    pl_in = nc.dram_tensor("pl_in", [T, 3], F32)
    pl_out = nc.dram_tensor("pl_out", [N_CORES * T, 3], F32, addr_space="Shared")
    ln_in = nc.dram_tensor("ln_in", [SB, T], BF16)
    ln_out = nc.dram_tensor("ln_out", [N_CORES * SB, T], BF16, addr_space="Shared")
    RG = [list(range(N_CORES))]

    with ExitStack() as ctx:
        tc = ctx.enter_context(tile.TileContext(nc))
        persist = ctx.enter_context(tc.tile_pool(name="persist", bufs=1))
        wpool = ctx.enter_context(tc.tile_pool(name="wpool", bufs=2))
        opool = ctx.enter_context(tc.tile_pool(name="opool", bufs=2))
        small = ctx.enter_context(tc.tile_pool(name="small", bufs=2))
        scratch = ctx.enter_context(tc.tile_pool(name="scratch", bufs=2))
        att = ctx.enter_context(tc.tile_pool(name="att", bufs=2))
        psum = ctx.enter_context(tc.tile_pool(name="psum", bufs=2, space="PSUM"))
        psum3 = ctx.enter_context(tc.tile_pool(name="psum3", bufs=3, space="PSUM"))
        psum1 = ctx.enter_context(tc.tile_pool(name="psum1", bufs=1, space="PSUM"))

        for _rep in range(reps):
            # ---------- persistent tiles ----------
            xt_sb = persist.tile([128, KT * NROW], BF16, tag="xt")
            fc_sb = persist.tile([128, RT * VSH], BF16, tag="fc")
            stats_parts = persist.tile([128, RT * CH], F32, tag="stp")
            a_sb = persist.tile([128, RT], F32, tag="asc")
            iota_v = persist.tile([128, VSH], F32, tag="iov")
            iota_k = persist.tile([128, 128], F32, tag="iok")
            idn_bf = persist.tile([128, 128], BF16, tag="idnb")
            idn_f = persist.tile([128, 128], F32, tag="idnf")
            einv_sb = persist.tile([128, B * 4 * kp_t], F32, tag="einv")
            gcol_sb = persist.tile([128, B * kp_t], F32, tag="gcol")
            ct_sb = persist.tile([128, B * kp_t * T], BF16, tag="ct")
            g_sb = persist.tile([128, B * kp_t * VSH], BF16, tag="g")
            qT_sb = persist.tile([128, KT * T], BF16, tag="qT")
            kT_sb = persist.tile([128, KT * SB], BF16, tag="kT")
            src_sb = persist.tile([128, KT * SB], BF16, tag="srcT")
            xq_sb = persist.tile([128, KT * T], BF16, tag="xq")
            yaug_sb = persist.tile([128, 2 * 32], BF16, tag="yaug")
            expT_sb = persist.tile([128, 2 * H * T], BF16, tag="expT")
            plh_sb = persist.tile([128, 2 * T], F32, tag="plh")
            cent_sb = persist.tile([128, 2 * SB], F32, tag="cent")
            rinv_att = persist.tile([128, 2], F32, tag="rinva")
            lnsc_sb = persist.tile([128, 2 * SB], BF16, tag="lnsc")
            lnT_sb = persist.tile([128, 2 * T], BF16, tag="lnT")
            p0_all = persist.tile([128, RT], F32, tag="p0")
            pj_sb = persist.tile([128, 2], F32, tag="pj")
            km_sb = persist.tile([128, 2], F32, tag="km")
            bsel_sb = persist.tile([128, B * 2], F32, tag="bsel")
            wq_sb = persist.tile([128, KT * D], BF16, tag="wq")
            wk_sb = persist.tile([128, KT * D], BF16, tag="wk")
            am_sb = persist.tile([128, KT * 32], BF16, tag="am")
            wpx_sb = persist.tile([128, KT * 3], BF16, tag="wpx")

            ATT = stage >= 2

            # ---------- loads / constants ----------
            nc.sync.dma_start(out=xq_sb[:], in_=xqT)
            nc.sync.dma_start(out=src_sb[:], in_=srcT)
            nc.sync.dma_start(out=wq_sb[:], in_=WqT)
            nc.sync.dma_start(out=wk_sb[:], in_=WkT)
            nc.sync.dma_start(out=am_sb[:], in_=Amat)
            nc.sync.dma_start(out=xt_sb[:], in_=xT)
            nc.sync.dma_start(out=wpx_sb[:], in_=WpxT)
            nc.sync.dma_start(out=km_sb[:], in_=kmask)
            nc.sync.dma_start(out=bsel_sb[:], in_=bsel)
            nc.sync.dma_start(out=einv_sb[:], in_=EInv)
            nc.sync.dma_start(out=gcol_sb[:], in_=Gcol)
            nc.vector.memset(plh_sb[:], 0.0)
            nc.gpsimd.iota(iota_v[:], [[1, VSH]], channel_multiplier=0,
                           allow_small_or_imprecise_dtypes=True)
            nc.gpsimd.iota(iota_k[:], [[1, 128]], channel_multiplier=0,
                           allow_small_or_imprecise_dtypes=True)
            io2 = scratch.tile([128, 128], F32, tag="io2")
            nc.gpsimd.iota(io2[:], [[0, 128]], channel_multiplier=1,
                           allow_small_or_imprecise_dtypes=True)
            nc.vector.tensor_tensor(out=idn_bf[:], in0=iota_k[:], in1=io2[:],
                                    op=ALU.is_equal)
            nc.vector.tensor_tensor(out=idn_f[:], in0=iota_k[:], in1=io2[:],
                                    op=ALU.is_equal)

            # ---------- G one-hot (independent of everything else) ----------
            for b in range(B if stage >= 5 else 0):
                for kpi in range(kp_t):
                    gcol = b * kp_t + kpi
                    nc.gpsimd.tensor_scalar(
                        out=g_sb[:, gcol * VSH:(gcol + 1) * VSH], in0=iota_v[:],
                        scalar1=gcol_sb[:, gcol:gcol + 1], scalar2=None,
                        op0=ALU.is_equal)

            # ---------- attention projections ----------
            for m in range(KT if ATT else 0):
                psq = psum.tile([128, T], F32, tag="at")
                for k in range(KT):
                    nc.tensor.matmul(psq[:],
                                     wq_sb[:, k * D + m * 128: k * D + (m + 1) * 128],
                                     xq_sb[:, k * T:(k + 1) * T],
                                     start=(k == 0), stop=(k == KT - 1))
                nc.scalar.activation(qT_sb[:, m * T:(m + 1) * T], psq[:], AF.Copy)
                psk = psum.tile([128, SB], F32, tag="at")
                for k in range(KT):
                    nc.tensor.matmul(psk[:],
                                     wk_sb[:, k * D + m * 128: k * D + (m + 1) * 128],
                                     src_sb[:, k * SB:(k + 1) * SB],
                                     start=(k == 0), stop=(k == KT - 1))
                nc.scalar.activation(kT_sb[:, m * SB:(m + 1) * SB], psk[:], AF.Copy)

            # ---------- Yaug ----------
            for sh in range(2 if ATT else 0):
                psy = psum.tile([128, 32], F32, tag="at")
                for k in range(KT):
                    nc.tensor.matmul(psy[:],
                                     src_sb[:, k * SB + sh * 128: k * SB + (sh + 1) * 128],
                                     am_sb[:, k * 32:(k + 1) * 32],
                                     start=(k == 0), stop=(k == KT - 1))
                yd = yaug_sb[:, sh * 32:(sh + 1) * 32]
                nc.vector.tensor_copy(yd, psy[:])
                for h in range(H):
                    nc.vector.memset(yaug_sb[:, sh * 32 + h * 4 + 3: sh * 32 + h * 4 + 4], 1.0)
                nc.vector.tensor_scalar(out=yd, in0=yd, scalar1=km_sb[:, sh:sh + 1],
                                        scalar2=None, op0=ALU.mult)

            # ---------- per-head scoresT -> exp -> [N_h; d_h] ----------
            for h in range(H if ATT else 0):
                mt, po = h // 2, (h % 2) * 64
                plp = psum1.tile([4, T], F32, tag="pl")
                for sh in range(2):
                    ssc = psum.tile([128, T], F32, tag="at")
                    nc.tensor.matmul(
                        ssc[:],
                        kT_sb[po:po + 64, mt * SB + sh * 128: mt * SB + (sh + 1) * 128],
                        qT_sb[po:po + 64, mt * T:(mt + 1) * T],
                        start=True, stop=True)
                    ed = expT_sb[:, (h * 2 + sh) * T:(h * 2 + sh + 1) * T]
                    nc.scalar.activation(ed, ssc[:], AF.Exp, scale=float(DH ** -0.5))
                    nc.tensor.matmul(plp[:],
                                     yaug_sb[:, sh * 32 + h * 4: sh * 32 + h * 4 + 4],
                                     ed, start=(sh == 0), stop=(sh == 1))
                gq, gm = h // 4, h % 4
                nc.scalar.activation(
                    plh_sb[32 * gm:32 * gm + 4, gq * T:(gq + 1) * T], plp[:], AF.Copy)

            # ---------- per-head divide, c@Z partials, AllGather #1 ----------
            for th in range(2 if ATT else 0):
                cacc = att.tile([128, 3], F32, tag="cacc")
                nc.vector.memset(cacc[:], 0.0)
                for gq in range(2):
                    ptp = psum.tile([128, 128], F32, tag="at")
                    nc.tensor.transpose(
                        ptp[:], plh_sb[:, gq * T + th * 128: gq * T + (th + 1) * 128],
                        idn_f[:])
                    pt = att.tile([128, 128], F32, tag="pt")
                    nc.vector.tensor_copy(pt[:], ptp[:])
                    for gm in range(4):
                        rh = small.tile([128, 1], F32, tag="rh")
                        nc.vector.reciprocal(rh[:], pt[:, 32 * gm + 3: 32 * gm + 4])
                        nc.vector.scalar_tensor_tensor(
                            out=cacc[:], in0=pt[:, 32 * gm: 32 * gm + 3], scalar=rh[:],
                            in1=cacc[:], op0=ALU.mult, op1=ALU.add)
                nc.sync.dma_start(out=pl_in.ap()[th * 128:(th + 1) * 128, :],
                                  in_=cacc[:])
            if stage >= 3:
                if no_coll:
                    nc.sync.dma_start(out=pl_out.ap()[0:T, :], in_=pl_in.ap())
                else:
                    nc.gpsimd.collective_compute(
                        "AllGather", ALU.bypass, replica_groups=RG,
                        ins=[pl_in.ap().opt()], outs=[pl_out.ap().opt()])

            # ---------- copy path: scores_sum [t, s], LN center + rinv ----------
            for th in range(2 if ATT else 0):
                pss = psum.tile([128, SB], F32, tag="at")
                for k in range(KT):
                    nc.tensor.matmul(pss[:],
                                     qT_sb[:, k * T + th * 128: k * T + (th + 1) * 128],
                                     kT_sb[:, k * SB:(k + 1) * SB],
                                     start=(k == 0), stop=(k == KT - 1))
                msum = small.tile([128, 1], F32, tag="msum")
                nc.vector.tensor_reduce(out=msum[:], in_=pss[:],
                                        axis=mybir.AxisListType.X, op=ALU.add)
                mmean = small.tile([128, 1], F32, tag="mmean")
                nc.vector.tensor_scalar(out=mmean[:], in0=msum[:],
                                        scalar1=1.0 / SB, scalar2=None, op0=ALU.mult)
                cd = cent_sb[:, th * SB:(th + 1) * SB]
                nc.vector.tensor_scalar(out=cd, in0=pss[:], scalar1=mmean[:],
                                        scalar2=None, op0=ALU.subtract)
                c2 = scratch.tile([128, SB], F32, tag="c2")
                vsum = small.tile([128, 1], F32, tag="vsum")
                nc.vector.scalar_tensor_tensor(out=c2[:], in0=cd, scalar=1.0,
                                               in1=cd, op0=ALU.mult, op1=ALU.mult,
                                               accum_out=vsum[:])
                rr = _rsqrt_cols(nc, small, vsum[:], 1.0 / SB, 1e-5, "ra")
                nc.vector.tensor_copy(rinv_att[:, th:th + 1], rr[:])

            # ---------- p assembly (needs pl_out) ----------
            for b in range(B if stage >= 3 else 0):
                for th in range(2):
                    r = b * 2 + th
                    plx = psum.tile([128, 3], F32, tag="at")
                    for k in range(KT):
                        nc.tensor.matmul(
                            plx[:],
                            xt_sb[:, k * NROW + b * T + th * 128: k * NROW + b * T + (th + 1) * 128],
                            wpx_sb[:, k * 3:(k + 1) * 3],
                            start=(k == 0), stop=(k == KT - 1))
                    cz1 = att.tile([128, 3], F32, tag="cz1")
                    cz2 = att.tile([128, 3], F32, tag="cz2")
                    nc.sync.dma_start(
                        out=cz1[:],
                        in_=pl_out.ap()[(2 * b) * T + th * 128: (2 * b) * T + (th + 1) * 128, :])
                    nc.sync.dma_start(
                        out=cz2[:],
                        in_=pl_out.ap()[(2 * b + 1) * T + th * 128: (2 * b + 1) * T + (th + 1) * 128, :])
                    l3 = small.tile([128, 3], F32, tag="l3")
                    nc.vector.tensor_tensor(out=l3[:], in0=plx[:], in1=cz1[:], op=ALU.add)
                    nc.vector.tensor_tensor(out=l3[:], in0=l3[:], in1=cz2[:], op=ALU.add)
                    mx = small.tile([128, 1], F32, tag="mx")
                    nc.vector.tensor_reduce(out=mx[:], in_=l3[:],
                                            axis=mybir.AxisListType.X, op=ALU.max)
                    lc = small.tile([128, 3], F32, tag="lc")
                    nc.vector.tensor_scalar(out=lc[:], in0=l3[:], scalar1=mx[:],
                                            scalar2=None, op0=ALU.subtract)
                    pe = small.tile([128, 3], F32, tag="pe")
                    nc.scalar.activation(pe[:], lc[:], AF.Exp)
                    se = small.tile([128, 1], F32, tag="se")
                    nc.vector.tensor_reduce(out=se[:], in_=pe[:],
                                            axis=mybir.AxisListType.X, op=ALU.add)
                    rs = small.tile([128, 1], F32, tag="rs")
                    nc.vector.reciprocal(rs[:], se[:])
                    p3 = small.tile([128, 3], F32, tag="p3")
                    nc.vector.tensor_scalar(out=p3[:], in0=pe[:], scalar1=rs[:],
                                            scalar2=None, op0=ALU.mult)
                    nc.vector.tensor_copy(p0_all[:, r:r + 1], p3[:, 0:1])
                    if b == 0:
                        nc.vector.tensor_scalar(out=pj_sb[:, th:th + 1],
                                                in0=p3[:, 1:2],
                                                scalar1=bsel_sb[:, 0:1],
                                                scalar2=None, op0=ALU.mult)
                    else:
                        nc.vector.scalar_tensor_tensor(
                            out=pj_sb[:, th:th + 1], in0=p3[:, 1:2],
                            scalar=bsel_sb[:, 2 * b:2 * b + 1],
                            in1=pj_sb[:, th:th + 1], op0=ALU.mult, op1=ALU.add)
                    nc.vector.scalar_tensor_tensor(
                        out=pj_sb[:, th:th + 1], in0=p3[:, 2:3],
                        scalar=bsel_sb[:, 2 * b + 1:2 * b + 2],
                        in1=pj_sb[:, th:th + 1], op0=ALU.mult, op1=ALU.add)

            # ---------- scale + transpose copy rows, AllGather #2 ----------
            for th in range(2 if stage >= 4 else 0):
                scl = small.tile([128, 1], F32, tag="scl")
                nc.vector.tensor_tensor(out=scl[:], in0=rinv_att[:, th:th + 1],
                                        in1=pj_sb[:, th:th + 1], op=ALU.mult)
                nc.vector.tensor_scalar(out=lnsc_sb[:, th * SB:(th + 1) * SB],
                                        in0=cent_sb[:, th * SB:(th + 1) * SB],
                                        scalar1=scl[:], scalar2=None, op0=ALU.mult)
            for sh in range(2 if stage >= 4 else 0):
                for th in range(2):
                    ptt = psum.tile([128, 128], BF16, tag="at")
                    nc.tensor.transpose(
                        ptt[:], lnsc_sb[:, th * SB + sh * 128: th * SB + (sh + 1) * 128],
                        idn_bf[:])
                    nc.vector.tensor_copy(
                        lnT_sb[:, sh * T + th * 128: sh * T + (th + 1) * 128], ptt[:])
                nc.sync.dma_start(out=ln_in.ap()[sh * 128:(sh + 1) * 128, :],
                                  in_=lnT_sb[:, sh * T:(sh + 1) * T])
            if stage >= 4:
                if no_coll:
                    nc.sync.dma_start(out=ln_out.ap()[0:SB, :], in_=ln_in.ap())
                else:
                    nc.gpsimd.collective_compute(
                        "AllGather", ALU.bypass, replica_groups=RG,
                        ins=[ln_in.ap().opt()], outs=[ln_out.ap().opt()])

            # ---------- Ct compaction per batch (needs ln_out) ----------
            for b in range(B if stage >= 5 else 0):
                for kpi in range(kp_t):
                    ctp = psum.tile([128, T], F32, tag="at")
                    for kt in range(4):
                        et = scratch.tile([128, 128], BF16, tag="et")
                        col = b * 4 * kp_t + kt * kp_t + kpi
                        nc.vector.tensor_scalar(out=et[:], in0=iota_k[:],
                                                scalar1=einv_sb[:, col:col + 1],
                                                scalar2=None, op0=ALU.is_equal)
                        lng = att.tile([128, T], BF16, tag="lng")
                        nc.sync.dma_start(
                            out=lng[:],
                            in_=ln_out.ap()[b * 2 * SB + kt * 128: b * 2 * SB + (kt + 1) * 128, :])
                        nc.tensor.matmul(ctp[:], et[:], lng[:],
                                         start=(kt == 0), stop=(kt == 3))
                    nc.vector.tensor_copy(
                        ct_sb[:, (b * kp_t + kpi) * T:(b * kp_t + kpi + 1) * T], ctp[:])

            # ---------- fc matmul + stats + scatter/epilogue, pipelined ----------
            def emit_fc_chunk(g, c):
                r0, nr = GROUPS[g]
                wt = wpool.tile([128, KT * CW], BF16, tag="w")
                nc.sync.dma_start(out=wt[:], in_=Wsw[c])
                for r in range(r0, r0 + nr):
                    ps = psum3.tile([128, CW], F32, tag="fcps")
                    for k in range(KT):
                        nc.tensor.matmul(
                            ps[:],
                            xt_sb[:, k * NROW + r * 128: k * NROW + (r + 1) * 128],
                            wt[:, k * CW:(k + 1) * CW],
                            start=(k == 0), stop=(k == KT - 1))
                    dst = fc_sb[:, r * VSH + c * CW: r * VSH + (c + 1) * CW]
                    nc.scalar.activation(dst, ps[:], AF.Copy)
                    sq = scratch.tile([128, CW], BF16, tag="sq")
                    nc.vector.scalar_tensor_tensor(
                        out=sq[:], in0=dst, scalar=1.0, in1=dst,
                        op0=ALU.mult, op1=ALU.mult,
                        accum_out=stats_parts[:, r * CH + c: r * CH + c + 1])

            def emit_stats_ar(g):
                r0, nr = GROUPS[g]
                stats_g = small.tile([128, nr], F32, tag=f"sts{g}")
                nc.vector.tensor_reduce(
                    out=stats_g[:],
                    in_=stats_parts[:, r0 * CH:(r0 + nr) * CH].rearrange(
                        "p (r c) -> p r c", c=CH),
                    axis=mybir.AxisListType.X, op=ALU.add)
                nc.sync.dma_start(out=st_in[g].ap(), in_=stats_g[:])
                if no_coll:
                    nc.sync.dma_start(out=st_out[g].ap(), in_=st_in[g].ap())
                else:
                    nc.gpsimd.collective_compute(
                        "AllReduce", ALU.add, replica_groups=RG,
                        ins=[st_in[g].ap().opt()], outs=[st_out[g].ap().opt()])

            def emit_a(g):
                r0, nr = GROUPS[g]
                gst = small.tile([128, nr], F32, tag=f"gst{g}")
                nc.sync.dma_start(out=gst[:], in_=st_out[g].ap())
                rfc = _rsqrt_cols(nc, small, gst[:], 1.0 / V, 1e-5, f"rf{g}")
                if stage >= 3:
                    nc.vector.tensor_tensor(
                        out=a_sb[:, r0:r0 + nr],
                        in0=p0_all[:, r0:r0 + nr], in1=rfc[:], op=ALU.mult)
                else:
                    nc.vector.tensor_copy(a_sb[:, r0:r0 + nr], rfc[:])

            def emit_epilogue_rowtile(r):
                b, th = r // 2, r % 2
                diag_a = scratch.tile([128, 128], BF16, tag="diag")
                nc.vector.tensor_scalar(out=diag_a[:], in0=idn_bf[:],
                                        scalar1=a_sb[:, r:r + 1],
                                        scalar2=None, op0=ALU.mult)
                # corr = a / bf16(a): f32 refinement applied via the ACT scale
                ab = small.tile([128, 1], BF16, tag="ab")
                nc.vector.tensor_copy(ab[:], a_sb[:, r:r + 1])
                abf = small.tile([128, 1], F32, tag="abf")
                nc.vector.tensor_copy(abf[:], ab[:])
                rab = small.tile([128, 1], F32, tag="rab")
                nc.vector.reciprocal(rab[:], abf[:])
                corr = small.tile([128, 1], F32, tag="corr")
                nc.vector.tensor_tensor(out=corr[:], in0=a_sb[:, r:r + 1],
                                        in1=rab[:], op=ALU.mult)
                ot = opool.tile([128, VSH], F32, tag="ot")
                for c in range(CH):
                    psc = psum.tile([128, CW], F32, tag="scps")
                    if stage >= 5:
                        for kpi in range(kp_t):
                            nc.tensor.matmul(
                                psc[:],
                                ct_sb[:, (b * kp_t + kpi) * T + th * 128:(b * kp_t + kpi) * T + (th + 1) * 128],
                                g_sb[:, (b * kp_t + kpi) * VSH + c * CW:(b * kp_t + kpi) * VSH + (c + 1) * CW],
                                start=(kpi == 0), stop=False)
                    nc.tensor.matmul(
                        psc[:], diag_a[:],
                        fc_sb[:, r * VSH + c * CW: r * VSH + (c + 1) * CW],
                        start=(stage < 5), stop=True)
                    nc.scalar.activation(
                        ot[:, c * CW:(c + 1) * CW], psc[:], AF.Copy,
                        scale=corr[:])
                nc.sync.dma_start(out=out[r * 128:(r + 1) * 128, :], in_=ot[:])

            for c in range(CH):
                emit_fc_chunk(0, c)
            emit_stats_ar(0)
            emit_a(0)
            for c in range(CH):
                emit_fc_chunk(1, c)
                if c >= CH - GROUPS[0][1]:   # interleave group-0 epilogue
                    emit_epilogue_rowtile(c - (CH - GROUPS[0][1]))
            emit_stats_ar(1)
            emit_a(1)
            for r in range(GROUPS[1][0], GROUPS[1][0] + GROUPS[1][1]):
                emit_epilogue_rowtile(r)

    nc.compile()
    return nc


def _swz(a):
    """[D, N] -> [128, KT*N] bf16 swizzle: row k*128+p -> partition p, col block k."""
    Dd, n = a.shape
    kt = Dd // 128
    return np.ascontiguousarray(
        a.reshape(kt, 128, n).transpose(1, 0, 2).reshape(128, kt * n)).astype(BF)


def host_prep(inputs):
    g = {k: np.asarray(v) for k, v in inputs.items()}
    x = g['tgt_dec_out'].astype(np.float32).reshape(NROW, D)
    Wfc = g['Wfc'].astype(np.float32)

    Wc = Wfc - Wfc.mean(axis=0, keepdims=True)
    Wext = np.zeros((VEXT, D), np.float32)
    Wext[:V] = Wc
    WextT = Wext.T

    xT_sw = _swz(x.T)
    Wp = g['Wp'].astype(np.float32)
    WpxT_sw = _swz(Wp[:, :D].T)

    maps = [g['src1_map_idx'].astype(np.int64), g['src2_map_idx'].astype(np.int64)]
    keys = [g['src1_key'].astype(np.float32), g['src2_key'].astype(np.float32)]

    counts = np.zeros((N_CORES, B), np.int32)
    for b in range(B):
        for j in range(2):
            cs, ns = np.unique(maps[j][b] // VSH, return_counts=True)
            counts[cs, b] += ns.astype(np.int32)
    kp = max(128, int(np.ceil(counts.max() / 128.0)) * 128)
    kp_t = kp // 128

    in_maps = []
    for core in range(N_CORES):
        bc, jc = core // 2, core % 2
        Wq = g[f'Wq{jc + 1}'].astype(np.float32)
        Wk = g[f'Wk{jc + 1}'].astype(np.float32)
        Wv = g[f'Wv{jc + 1}'].astype(np.float32)
        Wo = g[f'Wo{jc + 1}'].astype(np.float32)
        Z = Wo.T @ Wp[:, D * (jc + 1): D * (jc + 2)].T
        A = np.zeros((D, 32), np.float32)
        for h in range(H):
            A[:, h * 4: h * 4 + 3] = Wv[h * DH:(h + 1) * DH, :].T @ Z[h * DH:(h + 1) * DH, :]
        src = keys[jc][bc]
        km = np.sign(np.abs(src).sum(-1)).astype(np.float32)
        bsel = np.zeros((B, 2), np.float32)
        bsel[bc, jc] = 1.0

        lo = core * VSH
        einv = np.full((B, 4 * kp_t, 128), -1, np.float32)
        gcolv = np.full((B, kp_t, 128), -1, np.float32)
        for b in range(B):
            slot = 0
            for j in range(2):
                mrow = maps[j][b]
                for s in range(SB):
                    m = int(mrow[s])
                    if lo <= m < lo + VSH:
                        sglob = j * SB + s
                        kt, p = sglob // 128, sglob % 128
                        kpi, mloc = slot // 128, slot % 128
                        einv[b, kt * kp_t + kpi, p] = mloc
                        gcolv[b, kpi, mloc] = m - lo
                        slot += 1
        EInv = np.ascontiguousarray(einv.reshape(B * 4 * kp_t, 128).T)
        Gcol = np.ascontiguousarray(gcolv.reshape(B * kp_t, 128).T)

        Wsw = np.empty((CH, 128, KT * CW), BF)
        WT_sh = WextT[:, lo:lo + VSH]
        for c in range(CH):
            Wsw[c] = _swz(WT_sh[:, c * CW:(c + 1) * CW])

        in_maps.append({
            "xT": xT_sw,
            "Wsw": Wsw,
            "xqT": _swz(x.reshape(B, T, D)[bc].T),
            "srcT": _swz(src.T),
            "WqT": _swz(Wq.T),
            "WkT": _swz(Wk.T),
            "Amat": _swz(A),
            "WpxT": WpxT_sw,
            "kmask": np.ascontiguousarray(km.reshape(2, 128).T).astype(np.float32),
            "bsel": np.broadcast_to(bsel.reshape(1, B * 2), (128, B * 2)).copy(),
            "EInv": EInv,
            "Gcol": Gcol,
        })
    return in_maps, kp_t


class SpmdRunner:
    """Builds the shard_map-jitted bass executable once; reusable across calls."""

    def __init__(self, nc, n_cores):
        bass2jax.install_neuronx_cc_hook()
        self.n_cores = n_cores
        part_name = nc.partition_id_tensor.name if nc.partition_id_tensor else None
        in_names, out_names, out_avals, zero_outs = [], [], [], []
        for alloc in nc.m.functions[0].allocations:
            if not isinstance(alloc, mybir.MemoryLocationSet):
                continue
            name = alloc.memorylocations[0].name
            if alloc.kind == "ExternalInput":
                if name != part_name:
                    in_names.append(name)
            elif alloc.kind == "ExternalOutput":
                shape = tuple(alloc.tensor_shape)
                dtype = mybir.dt.np(alloc.dtype)
                out_names.append(name)
                out_avals.append(jax.core.ShapedArray(shape, dtype))
                zero_outs.append(np.zeros(shape, dtype))
        self.in_names, self.out_names = in_names, out_names
        self.out_avals, self.zero_outs = out_avals, zero_outs
        n_params, n_outs = len(in_names), len(out_names)
        all_names = in_names + out_names
        if part_name is not None:
            all_names = all_names + [part_name]

        def _body(*args):
            operands = list(args)
            if part_name is not None:
                operands.append(bass2jax.partition_id_tensor())
            outs = bass2jax._bass_exec_p.bind(
                *operands,
                out_avals=tuple(out_avals),
                in_names=tuple(all_names),
                out_names=tuple(out_names),
                lowering_input_output_aliases=(),
                sim_require_finite=True,
                sim_require_nnan=True,
                nc=nc,
            )
            return tuple(outs)

        devices = jax.devices()[:n_cores]
        self.mesh = Mesh(np.asarray(devices), ("core",))
        in_specs = (PartitionSpec("core"),) * (n_params + n_outs)
        out_specs = (PartitionSpec("core"),) * n_outs
        self.jitted = jax.jit(
            shard_map(_body, mesh=self.mesh, in_specs=in_specs,
                      out_specs=out_specs, check_rep=False),
            keep_unused=True,
        )
        self.sharding = NamedSharding(self.mesh, PartitionSpec("core"))
        self._zs = None

    def concat_inputs(self, in_maps):
        return [np.concatenate([np.asarray(in_maps[c][n]) for c in range(self.n_cores)],
                               axis=0) for n in self.in_names]

    def zeros(self):
        if self._zs is None:
            self._zs = [jnp.zeros((self.n_cores * z.shape[0], *z.shape[1:]), z.dtype,
                                  device=self.sharding) for z in self.zero_outs]
        return self._zs

    def run(self, in_maps):
        outs = self.jitted(*self.concat_inputs(in_maps), *self.zeros())
        return [np.asarray(o) for o in outs]


def _numpy_reference(g):
    """Exact numpy fallback (only used if an impossible-input assumption is
    violated, e.g. nonzero biases; the problem generator always passes zeros)."""
    def ln(x):
        m = x.mean(-1, keepdims=True)
        v = ((x - m) ** 2).mean(-1, keepdims=True)
        return (x - m) / np.sqrt(v + 1e-5)

    x = g['tgt_dec_out'].astype(np.float64)
    out = np.zeros((B, T, VEXT))
    fc = x.reshape(NROW, D) @ g['Wfc'].astype(np.float64).T + g['bfc'].astype(np.float64)
    tgt = np.zeros((NROW, VEXT)); tgt[:, :V] = ln(fc)
    tgt = tgt.reshape(B, T, VEXT)
    copies, cs = [], []
    for j in (1, 2):
        Wq, bq = g[f'Wq{j}'].astype(np.float64), g[f'bq{j}'].astype(np.float64)
        Wk, bk = g[f'Wk{j}'].astype(np.float64), g[f'bk{j}'].astype(np.float64)
        Wv, bv = g[f'Wv{j}'].astype(np.float64), g[f'bv{j}'].astype(np.float64)
        Wo, bo = g[f'Wo{j}'].astype(np.float64), g[f'bo{j}'].astype(np.float64)
        key = g[f'src{j}_key'].astype(np.float64)
        mi = g[f'src{j}_map_idx'].astype(np.int64)
        qm = np.sign(np.abs(x).sum(-1))
        kmm = np.sign(np.abs(key).sum(-1))
        q = (x @ Wq.T + bq).reshape(B, T, H, DH).transpose(0, 2, 1, 3) * DH ** -0.5
        k = (key @ Wk.T + bk).reshape(B, SB, H, DH).transpose(0, 2, 1, 3)
        v = (key @ Wv.T + bv).reshape(B, SB, H, DH).transpose(0, 2, 1, 3)
        att = np.einsum('bhtd,bhkd->bhtk', q, k)
        oa = att * kmm[:, None, None, :]
        att = np.where((kmm == 0)[:, None, None, :], -np.inf, att)
        att = np.exp(att - att.max(-1, keepdims=True))
        att = att / att.sum(-1, keepdims=True)
        o = np.einsum('bhtk,bhkd->bhtd', att, v).transpose(0, 2, 1, 3).reshape(B, T, D)
        o = (o @ Wo.T + bo) * qm[:, :, None]
        oa = (oa * qm[:, None, :, None]).mean(1)
        cp = np.zeros((B, T, VEXT))
        lnoa = ln(oa)
        for b in range(B):
            for s in range(SB):
                cp[b, :, mi[b, s]] += lnoa[b, :, s]
        copies.append(cp); cs.append(o)
    Wp, bp = g['Wp'].astype(np.float64), g['bp'].astype(np.float64)
    lg = np.concatenate([x, cs[0], cs[1]], -1) @ Wp.T + bp
    e = np.exp(lg - lg.max(-1, keepdims=True)); p = e / e.sum(-1, keepdims=True)
    out = tgt * p[..., 0:1] + copies[0] * p[..., 1:2] + copies[1] * p[..., 2:3]
    return out.astype(np.float32)


def kernel(**inputs):
    g = {k: np.asarray(v) for k, v in inputs.items()}
    if any(np.any(g[b]) for b in ('bfc', 'bp', 'bq1', 'bk1', 'bv1', 'bo1',
                                  'bq2', 'bk2', 'bv2', 'bo2') if b in g):
        return _numpy_reference(g)
    in_maps, kp_t = host_prep(g)
    if kp_t not in _CACHE:
        nc = build_program(kp_t)
        _CACHE[kp_t] = SpmdRunner(nc, N_CORES)
    runner = _CACHE[kp_t]
    outs = runner.run(in_maps)
    full = outs[0].reshape(N_CORES, NROW, VSH)
    return np.concatenate(list(full), axis=1).reshape(B, T, VEXT).astype(np.float32)

